# revision 5
# baseline (speedup 1.0000x reference)
"""DimeNet-style GNN message passing on 8 Trainium2 NeuronCores.

Sharding: edges are packed into 128-edge "windows" such that each window's
triplet count <= K_FIX*128; windows are dealt to 8 cores (graph-parallel).
Each core owns its edges AND all triplets targeting them (gather and
scatter in the interaction block both use idx_kj, so triplet work is fully
local to the owning core).  Gather (x_kj[idx_kj]) is an expand-matmul with
a one-hot matrix; scatter-add is a matmul with the transposed one-hot,
accumulated in PSUM per window.  The only cross-core communication is one
ReduceScatter of the [H, N] atom-message partial sums.
"""
import os
import sys
import numpy as np

sys.path.insert(0, "/opt/trn_rl_repo")

H = 128
NR = 16
NS = 6
L = 2
CUTOFF = 8.0
NCORES = 8
TWO_PI = float(2 * np.pi)
F32 = np.float32
LAST_RESULTS = None


# ----------------------------------------------------------------------------
# host-side helpers
# ----------------------------------------------------------------------------

def _envelope(x):
    x5 = x ** 5
    return np.where(x < 1.0, 1.0 / x - 28.0 * x5 + 48.0 * x5 * x - 21.0 * x5 * x * x, 0.0)


def _pack_edges(deg, n_windows):
    """Deal edges (sorted by degree desc) snake-wise into n_windows windows.
    Returns list of edge-id lists. Balances both edge count and triplet sum."""
    order = np.argsort(-deg, kind="stable")
    wins = [[] for _ in range(n_windows)]
    i = 0
    fwd = True
    for e in order:
        w = i if fwd else n_windows - 1 - i
        wins[w].append(int(e))
        i += 1
        if i == n_windows:
            i = 0
            fwd = not fwd
    return wins


def kernel(**inputs):
    import time as _time
    _tt = {"t": _time.perf_counter()}

    def _mark(name):
        now = _time.perf_counter()
        print(f"[kernel] {name}: {now - _tt['t']:.2f}s", file=sys.stderr)
        _tt["t"] = now

    import concourse.bass as bass
    import concourse.bacc as bacc
    import concourse.mybir as mybir
    import concourse.tile as tile
    from concourse.bass import IndirectOffsetOnAxis
    from concourse.bass_utils import run_bass_kernel_spmd

    DT = mybir.dt.float32

    af = np.asarray(inputs["atom_feature"], F32)     # [N,133]
    ef = np.asarray(inputs["edge_feature"], F32)     # [E,14]
    dist = np.asarray(inputs["dist"], F32)           # [E]
    angle = np.asarray(inputs["angle"], F32)         # [T]
    i_idx = np.asarray(inputs["i"]).astype(np.int64)
    j_idx = np.asarray(inputs["j"]).astype(np.int64)
    idx_kj = np.asarray(inputs["idx_kj"]).astype(np.int64)
    ib_eid = np.asarray(inputs["incomebond_edge_ids"]).astype(np.int64)
    ib_atom = np.asarray(inputs["incomebond_index_to_atom"]).astype(np.int64)

    N, FA = af.shape
    E = ef.shape[0]
    T = angle.shape[0]
    FE = ef.shape[1]
    FI = FA + FE                                     # 147

    # --- host precompute (index decode / input gathers / tiny per-edge scalars)
    atom_type = np.argmax(af[:, :100], axis=1)
    x_emb = np.asarray(inputs["emb_table"], F32)[atom_type]          # [N,H]
    d_edge = (dist / CUTOFF).astype(F32)                             # [E]
    env_edge = _envelope(d_edge.astype(np.float64)).astype(F32)      # [E]

    # --- edge -> window packing
    deg = np.bincount(idx_kj, minlength=E)
    # windows total: multiple of 32 (so NW per core is a multiple of 4) and
    # enough that the average triplets/window leaves headroom under 512
    NW_TOT = -(-(-(-E // 128)) // 32) * 32
    while T / NW_TOT > 490.0:
        NW_TOT += 32
    wins = _pack_edges(deg, NW_TOT)
    tmax = max(int(deg[w].sum()) for w in wins if w)
    K_FIX = max(1, -(-tmax // 128))
    TPW = 128 * K_FIX                                # triplet slots per window
    NW = NW_TOT // NCORES                            # windows per core
    EC = NW * 128                                    # edge slots per core
    NSC = -(-NW // 4)                                # superchunks of 4 windows
    assert NSC * 4 == NW, (NW,)

    # deal windows to cores (snake by triplet load)
    wloads = np.array([int(deg[w].sum()) for w in wins])
    worder = np.argsort(-wloads, kind="stable")
    core_wins = [[] for _ in range(NCORES)]
    i = 0
    fwd = True
    for w in worder:
        c = i if fwd else NCORES - 1 - i
        core_wins[c].append(int(w))
        i += 1
        if i == NCORES:
            i = 0
            fwd = not fwd

    # triplets grouped by target edge
    t_order = np.argsort(idx_kj, kind="stable")
    t_sorted_edge = idx_kj[t_order]
    seg_starts = np.searchsorted(t_sorted_edge, np.arange(E))
    seg_ends = np.searchsorted(t_sorted_edge, np.arange(E), side="right")

    owner = np.full(E, -1, np.int32)
    localrow = np.full(E, -1, np.int32)

    per_core = []
    for c in range(NCORES):
        edge_ids = np.full(EC, -1, np.int64)
        for wl, w in enumerate(core_wins[c]):
            es = wins[w]
            edge_ids[wl * 128: wl * 128 + len(es)] = es
        real = edge_ids >= 0
        re = edge_ids[real]
        owner[re] = c
        localrow[re] = np.nonzero(real)[0].astype(np.int32)

        # per-edge device inputs (feature-major, padded edges -> 0)
        ibT = np.zeros((FI, EC), F32)
        embiT = np.zeros((H, EC), F32)
        embjT = np.zeros((H, EC), F32)
        dE = np.full((1, EC), 0.5, F32)
        envE = np.zeros((1, EC), F32)
        ibT[:FA, real] = af[j_idx[re]].T
        ibT[FA:, real] = ef[re].T
        embiT[:, real] = x_emb[i_idx[re]].T
        embjT[:, real] = x_emb[j_idx[re]].T
        dE[0, real] = d_edge[re]
        envE[0, real] = env_edge[re]

        # triplet slots
        TP = NW * TPW
        tripmeta = np.zeros((4, TP), F32)            # angle, dkj, envkj, segrel
        tripmeta[1] = 0.5
        tripmeta[3] = -1.0
        for wl in range(NW):
            pos = wl * TPW
            for p in range(128):
                e = edge_ids[wl * 128 + p]
                if e < 0:
                    continue
                ts = t_order[seg_starts[e]:seg_ends[e]]
                n = len(ts)
                if n == 0:
                    continue
                tripmeta[0, pos:pos + n] = angle[ts]
                tripmeta[1, pos:pos + n] = d_edge[e]
                tripmeta[2, pos:pos + n] = env_edge[e]
                tripmeta[3, pos:pos + n] = float(p)
                pos += n
            assert pos <= (wl + 1) * TPW
        # segrel transposed into columns of 128 for the scatter one-hot
        segcolT = np.ascontiguousarray(
            tripmeta[3].reshape(NW * K_FIX, 128).T)   # [128, NW*K_FIX]
        # one row per window: [angle | dkj | envkj | segrel] concatenated
        tripcat = np.ascontiguousarray(
            tripmeta.reshape(4, NW, TPW).transpose(1, 0, 2).reshape(1, NW * 4 * TPW))
        per_core.append(dict(ibT=ibT, embiT=embiT, embjT=embjT, dE=dE, envE=envE,
                             tripcat=tripcat, segcolT=segcolT,
                             edge_ids=edge_ids))

    # --- income bonds -> owner of source edge, laid out by target-atom window
    # atom windows: multiple of 32 so each core's final shard is a multiple of 512
    NAW = -(-(-(-N // 128)) // 32) * 32
    NA = NAW * 128                                   # padded atom count
    ASH = NA // NCORES                               # atoms per core for final
    bond_owner = owner[ib_eid]
    counts = np.zeros((NCORES, NAW), np.int64)
    for c in range(NCORES):
        sel = np.nonzero(bond_owner == c)[0]
        w_of = ib_atom[sel] // 128
        cnt = np.bincount(w_of, minlength=NAW)
        counts[c] = cnt
    K_A = max(1, -(-int(counts.max()) // 128))
    BPW = 128 * K_A
    BP = NAW * BPW
    for c in range(NCORES):
        srwar = np.zeros((1, BP), np.int32)
        tgw = np.full((1, BP), -1.0, F32)
        sel = np.nonzero(bond_owner == c)[0]
        aw = ib_atom[sel] // 128
        order2 = np.argsort(aw, kind="stable")
        sel = sel[order2]
        aw = aw[order2]
        starts = np.searchsorted(aw, np.arange(NAW))
        ends = np.searchsorted(aw, np.arange(NAW), side="right")
        for w in range(NAW):
            b = sel[starts[w]:ends[w]]
            n = len(b)
            srwar[0, w * BPW: w * BPW + n] = localrow[ib_eid[b]]
            tgw[0, w * BPW: w * BPW + n] = (ib_atom[b] - 128 * w).astype(F32)
        per_core[c]["srcrow"] = np.ascontiguousarray(srwar.reshape(NAW * K_A, 128).T)
        per_core[c]["tgtrel"] = np.ascontiguousarray(tgw.reshape(NAW * K_A, 128).T)
        afT = np.zeros((FA, ASH), F32)
        lo = c * ASH
        hi = min(N, lo + ASH)
        if hi > lo:
            afT[:, :hi - lo] = af[lo:hi].T
        per_core[c]["afT"] = afT

    # --- replicated weights / constants
    W = {k: np.asarray(v, F32) for k, v in inputs.items()
         if k not in ("atom_feature", "edge_feature", "dist", "angle", "i", "j",
                      "idx_kj", "idx_ji", "incomebond_edge_ids",
                      "incomebond_index_to_atom")}
    bf = W["bessel_freq"]                            # [NR] = pi*(1..NR)
    const = dict(
        ones512=np.ones((1, 512), F32),
        zeros512=np.zeros((128, 512), F32),
        q025=np.full((1, NS), 0.25, F32),
        svecn=(np.arange(NS, dtype=F32) / TWO_PI).reshape(1, NS),
        freqn=(bf / TWO_PI).reshape(1, NR).astype(F32),
        iota_mat=np.tile(np.arange(128, dtype=F32), (128, 1)),
        iota_col=np.arange(128, dtype=F32).reshape(128, 1),
        identity=np.eye(128, dtype=F32),
        Wi1a=W["W_i1_w"][:128], Wi1b=W["W_i1_w"][128:FI],
        b_i1=W["W_i1_b"].reshape(H, 1),
        Wrbf=W["lin_rbf_w"], b_rbf=W["lin_rbf_b"].reshape(H, 1),
        Wemb_i=W["lin_emb_w"][:H], Wemb_j=W["lin_emb_w"][H:2 * H],
        Wemb_r=W["lin_emb_w"][2 * H:], b_emb=W["lin_emb_b"].reshape(H, 1),
        Woa1=W["W_o_w"][:128], Woa2=W["W_o_w"][128:FA],
        Wom=W["W_o_w"][FA:], b_o=W["W_o_b"].reshape(H, 1),
        # REP6[r, s*16+r'] = delta(r,r');  REPC[s', s*16+r] = delta(s,s')
        REP6=np.tile(np.eye(NR, dtype=F32), (1, NS)),
        REPC=np.repeat(np.eye(NS, dtype=F32), NR, axis=1),
    )
    for l in range(L):
        const[f"Wkj{l}"] = W["L_kj_w"][l]
        const[f"b_kj{l}"] = W["L_kj_b"][l].reshape(H, 1)
        const[f"Wrbf2{l}"] = W["L_rbf2_w"][l]
        const[f"b_rbf2r{l}"] = W["L_rbf2_b"][l].reshape(1, H)
        const[f"Wsbf1{l}"] = W["L_sbf1_w"][l]
        const[f"Wsbf2{l}"] = W["L_sbf2_w"][l]
        const[f"Wdown{l}"] = W["L_down_w"][l]
        const[f"bdownr{l}"] = W["L_down_b"][l].reshape(1, H)
        const[f"Wup{l}"] = W["L_up_w"][l]
        const[f"bupr{l}"] = np.tile(W["L_up_b"][l].reshape(1, H), (1, K_FIX))
        const[f"Wres1_{l}"] = W["L_res1_w"][l]
        const[f"b_res1_{l}"] = W["L_res1_b"][l].reshape(H, 1)
        const[f"Wres2_{l}"] = W["L_res2_w"][l]
        const[f"b_res2_{l}"] = W["L_res2_b"][l].reshape(H, 1)

    _mark("host-prep")
    # ------------------------------------------------------------------
    # build the Bass program (identical for all cores)
    # ------------------------------------------------------------------
    nc = bacc.Bacc("TRN2", target_bir_lowering=False, debug=False,
                   num_devices=NCORES)

    def din(name, arr):
        return nc.dram_tensor(name, list(arr.shape), DT if arr.dtype == F32
                              else mybir.dt.int32, kind="ExternalInput")

    d_const = {k: din(k, v) for k, v in const.items()}
    p0 = per_core[0]
    d_ibT = din("ibT", p0["ibT"])
    d_embiT = din("embiT", p0["embiT"])
    d_embjT = din("embjT", p0["embjT"])
    d_dE = din("dE", p0["dE"])
    d_envE = din("envE", p0["envE"])
    d_tripcat = din("tripcat", p0["tripcat"])
    d_segcolT = din("segcolT", p0["segcolT"])
    d_srcrow = din("srcrow", p0["srcrow"])
    d_tgtrel = din("tgtrel", p0["tgtrel"])
    d_afT = din("afT", p0["afT"])
    d_out = nc.dram_tensor("outT", [H, ASH], DT, kind="ExternalOutput")

    TP = NW * TPW

    with tile.TileContext(nc) as tc:
        with (
            tc.tile_pool(name="const", bufs=1) as cpool,
            tc.tile_pool(name="sb", bufs=3) as sb,
            tc.tile_pool(name="sbsmall", bufs=3) as sbs,
            tc.tile_pool(name="psb", bufs=3, space="PSUM") as psb,      # [128,512]
            tc.tile_pool(name="pss", bufs=3, space="PSUM") as pss,      # [128,128]
            tc.tile_pool(name="psagg", bufs=2, space="PSUM") as psagg,  # agg
            tc.tile_pool(name="dram", bufs=1, space="DRAM") as dram,
        ):
            C = {}
            for k, v in const.items():
                t = cpool.tile(list(v.shape), DT, tag=k)
                nc.sync.dma_start(t[:], d_const[k][:])
                C[k] = t

            msg = [dram.tile([H, EC], DT, tag="msgA", name="msgA"),
                   dram.tile([H, EC], DT, tag="msgB", name="msgB")]
            rbfeT = dram.tile([H, EC], DT, tag="rbfeT")
            msgRM = dram.tile([EC, H], DT, tag="msgRM")
            apart = dram.tile([NCORES, H, ASH], DT, tag="apart")
            asum = dram.tile([H, ASH], DT, tag="asum")

            RELU = mybir.ActivationFunctionType.Relu
            SIN = mybir.ActivationFunctionType.Sin
            ADD = mybir.AluOpType.add
            MULT = mybir.AluOpType.mult
            ISEQ = mybir.AluOpType.is_equal
            MAX = mybir.AluOpType.max

            def sin_of_psum(p_arg, parts, width, tag):
                """p_arg holds arg/(2pi); returns SBUF tile sin(arg) [parts,width]."""
                qi = sbs.tile([parts, width], mybir.dt.int32, tag="sinqi")
                nc.vector.tensor_copy(qi[:], p_arg[:])
                qf = sbs.tile([parts, width], DT, tag="sinqf")
                nc.vector.tensor_copy(qf[:], qi[:])
                y = sbs.tile([parts, width], DT, tag="siny")
                nc.vector.scalar_tensor_tensor(y[:], qf[:], -1.0, p_arg[:], MULT, ADD)
                s = sbs.tile([parts, width], DT, tag="sins")
                nc.scalar.activation(s[:], y[:], SIN, scale=TWO_PI)
                return s

            # ---------------- phase 0: embedding ----------------
            for sc in range(NSC):
                cs = slice(sc * 512, sc * 512 + 512)
                ib_hi = sb.tile([128, 512], DT, tag="mt")
                nc.sync.dma_start(ib_hi[:], d_ibT[0:128, cs])
                ib_lo = sb.tile([FI - 128, 512], DT, tag="afl")
                nc.sync.dma_start(ib_lo[:], d_ibT[128:FI, cs])
                pm = psb.tile([128, 512], DT, tag="big")
                nc.tensor.matmul(pm[:], C["Wi1a"][:], ib_hi[:], start=True, stop=False)
                nc.tensor.matmul(pm[:], C["Wi1b"][:], ib_lo[:], start=False, stop=True)
                m0 = sb.tile([128, 512], DT, tag="s1")
                nc.vector.scalar_tensor_tensor(m0[:], pm[:], C["b_i1"][:, :1], C["zeros512"][:, :512], ADD, MAX)
                nc.sync.dma_start(msg[0][:, cs], m0[:])

                drow = sbs.tile([1, 512], DT, tag="ang_r")
                nc.sync.dma_start(drow[:], d_dE[:, cs])
                erow = sbs.tile([1, 512], DT, tag="env_r")
                nc.sync.dma_start(erow[:], d_envE[:, cs])
                parg = pss.tile([NR, 512], DT, tag="small")
                nc.tensor.matmul(parg[:], C["freqn"][:], drow[:], start=True, stop=True)
                sin16 = sin_of_psum(parg, NR, 512, "e")
                penv = pss.tile([NR, 512], DT, tag="small")
                nc.tensor.matmul(penv[:], C["ones512"][:, :NR], erow[:], start=True, stop=True)
                rbf0 = sb.tile([NR, 512], DT, tag="rbf16")
                nc.vector.tensor_tensor(rbf0[:], sin16[:], penv[:], op=MULT)
                prh = psb.tile([128, 512], DT, tag="big")
                nc.tensor.matmul(prh[:], C["Wrbf"][:], rbf0[:], start=True, stop=True)
                rbfh = sb.tile([128, 512], DT, tag="s2")
                nc.vector.scalar_tensor_tensor(rbfh[:], prh[:], C["b_rbf"][:, :1], C["zeros512"][:, :512], ADD, MAX)

                embi = sb.tile([128, 512], DT, tag="kj")
                nc.sync.dma_start(embi[:], d_embiT[:, cs])
                embj = sb.tile([128, 512], DT, tag="xkr")
                nc.sync.dma_start(embj[:], d_embjT[:, cs])
                pre = psb.tile([128, 512], DT, tag="big")
                nc.tensor.matmul(pre[:], C["Wemb_i"][:], embi[:], start=True, stop=False)
                nc.tensor.matmul(pre[:], C["Wemb_j"][:], embj[:], start=False, stop=False)
                nc.tensor.matmul(pre[:], C["Wemb_r"][:], rbfh[:], start=False, stop=True)
                rbe = sb.tile([128, 512], DT, tag="mnew")
                nc.vector.scalar_tensor_tensor(rbe[:], pre[:], C["b_emb"][:, :1], C["zeros512"][:, :512], ADD, MAX)
                nc.sync.dma_start(rbfeT[:, cs], rbe[:])

            # ---------------- phase 1: interaction layers ----------------
            for l in range([L, 0][os.environ.get("SKIP_P1") == "1"]):
                src, dst = msg[l % 2], msg[(l + 1) % 2]
                for sc in range(NSC):
                    cs = slice(sc * 512, sc * 512 + 512)
                    mt = sb.tile([128, 512], DT, tag="mt")
                    nc.sync.dma_start(mt[:], src[:, cs])
                    ret = sb.tile([128, 512], DT, tag="ret")
                    nc.sync.dma_start(ret[:], rbfeT[:, cs])
                    pkj = psb.tile([128, 512], DT, tag="big")
                    nc.tensor.matmul(pkj[:], C[f"Wkj{l}"][:], mt[:], start=True, stop=True)
                    kj = sb.tile([128, 512], DT, tag="kj")
                    nc.vector.scalar_tensor_tensor(kj[:], pkj[:], C[f"b_kj{l}"][:, :1], C["zeros512"][:, :512], ADD, MAX)
                    pr = psb.tile([128, 512], DT, tag="big")
                    nc.tensor.matmul(pr[:], C[f"b_rbf2r{l}"][:], C["ones512"][:],
                                     start=True, stop=False)
                    nc.tensor.matmul(pr[:], C[f"Wrbf2{l}"][:], ret[:], start=False, stop=True)
                    xkr = sb.tile([128, 512], DT, tag="xkr")
                    # xkr = relu(pr) * kj
                    nc.vector.scalar_tensor_tensor(xkr[:], pr[:], 0.0, kj[:], MAX, MULT)

                    for wi in range(4):
                        w = 4 * sc + wi
                        ws = slice(wi * 128, wi * 128 + 128)
                        # y = relu(xkr_w @ Wdown + b)   (row-major [e,f])
                        py = pss.tile([128, 128], DT, tag="small")
                        nc.tensor.matmul(py[:], C["ones512"][:, :128], C[f"bdownr{l}"][:],
                                         start=True, stop=False)
                        nc.tensor.matmul(py[:], xkr[:, ws], C[f"Wdown{l}"][:],
                                         start=False, stop=True)
                        y_rm = sb.tile([128, 128], DT, tag="y_rm")
                        nc.vector.tensor_scalar_max(y_rm[:], py[:], 0.0)

                        trow = sbs.tile([1, 4 * TPW], DT, tag="trow")
                        nc.sync.dma_start(trow[:], d_tripcat[:, w * 4 * TPW:(w + 1) * 4 * TPW])
                        ang_r = trow[:, 0:TPW]
                        dkj_r = trow[:, TPW:2 * TPW]
                        env_r = trow[:, 2 * TPW:3 * TPW]
                        seg_r = trow[:, 3 * TPW:4 * TPW]
                        segc = sbs.tile([128, K_FIX], DT, tag="segc")
                        nc.sync.dma_start(segc[:], d_segcolT[:, w * K_FIX:(w + 1) * K_FIX])

                        # sbf for this window: [NS*NR, TPW]
                        pa = pss.tile([NS, TPW], DT, tag="small")
                        nc.tensor.matmul(pa[:], C["q025"][:], C["ones512"][:, :TPW],
                                         start=True, stop=False)
                        nc.tensor.matmul(pa[:], C["svecn"][:], ang_r,
                                         start=False, stop=True)
                        cbf6 = sin_of_psum(pa, NS, TPW, "c")
                        pb = pss.tile([NR, TPW], DT, tag="small")
                        nc.tensor.matmul(pb[:], C["freqn"][:], dkj_r,
                                         start=True, stop=True)
                        sin16 = sin_of_psum(pb, NR, TPW, "t")
                        pe = pss.tile([NR, TPW], DT, tag="small")
                        nc.tensor.matmul(pe[:], C["ones512"][:, :NR], env_r,
                                         start=True, stop=True)
                        rbf16 = sbs.tile([NR, TPW], DT, tag="rbf16")
                        nc.vector.tensor_tensor(rbf16[:], sin16[:], pe[:], op=MULT)
                        pr96 = psb.tile([NS * NR, TPW], DT, tag="big")
                        nc.tensor.matmul(pr96[:], C["REP6"][:], rbf16[:], start=True, stop=True)
                        pc96 = psb.tile([NS * NR, TPW], DT, tag="big")
                        nc.tensor.matmul(pc96[:], C["REPC"][:], cbf6[:], start=True, stop=True)
                        cbf96 = sb.tile([NS * NR, TPW], DT, tag="cbf96")
                        nc.scalar.copy(cbf96[:], pc96[:])
                        sbf = sb.tile([NS * NR, TPW], DT, tag="sbf")
                        nc.vector.tensor_tensor(sbf[:], pr96[:], cbf96[:], op=MULT)

                        ps1 = psb.tile([128, TPW], DT, tag="big")
                        nc.tensor.matmul(ps1[:], C[f"Wsbf1{l}"][:], sbf[:], start=True, stop=True)
                        s1 = sb.tile([128, TPW], DT, tag="s1")
                        nc.vector.tensor_scalar_max(s1[:], ps1[:], 0.0)
                        ps2 = psb.tile([128, TPW], DT, tag="big")
                        nc.tensor.matmul(ps2[:], C[f"Wsbf2{l}"][:], s1[:], start=True, stop=True)
                        s2 = sb.tile([128, TPW], DT, tag="s2")
                        nc.vector.tensor_scalar_max(s2[:], ps2[:], 0.0)

                        # whole-window expand + multiply (N=TPW)
                        segb = sbs.tile([128, TPW], DT, tag="segb", bufs=2)
                        nc.gpsimd.partition_broadcast(segb[:], seg_r)
                        esub = sbs.tile([128, TPW], DT, tag="esub", bufs=2)
                        nc.vector.tensor_scalar(esub[:], segb[:],
                                                C["iota_col"][:, :1], None, ISEQ)
                        px = psb.tile([128, TPW], DT, tag="big")
                        for n0 in range(0, TPW, 512):
                            n1 = min(TPW, n0 + 512)
                            nc.tensor.matmul(px[:, n0:n1], y_rm[:], esub[:, n0:n1],
                                             start=True, stop=True)
                        xs = sb.tile([128, TPW], DT, tag="xs")
                        nc.vector.tensor_tensor(xs[:], px[:], s2[:], op=MULT)
                        # batched up-projection: all K_FIX chunks in one PSUM bank,
                        # one bias seed, one relu evict
                        pz = psb.tile([128, 128 * K_FIX], DT, tag="big")
                        for n0 in range(0, 128 * K_FIX, 512):
                            n1 = min(128 * K_FIX, n0 + 512)
                            nc.tensor.matmul(pz[:, n0:n1], C["ones512"][:, :128],
                                             C[f"bupr{l}"][:, n0:n1],
                                             start=True, stop=False)
                        for k in range(K_FIX):
                            ks = slice(k * 128, k * 128 + 128)
                            nc.tensor.matmul(pz[:, ks], xs[:, ks], C[f"Wup{l}"][:],
                                             start=False, stop=(k == K_FIX - 1))
                        z_rm = sb.tile([128, 128 * K_FIX], DT, tag="z_rm")
                        nc.vector.tensor_scalar_max(z_rm[:], pz[:], 0.0)
                        pagg = psagg.tile([128, 128], DT, tag="agg")
                        for k in range(K_FIX):
                            ks = slice(k * 128, k * 128 + 128)
                            # S_sub[t,e] = (segrel[t] == e)
                            ssub = sbs.tile([128, 128], DT, tag="ssub")
                            nc.vector.tensor_scalar(ssub[:], C["iota_mat"][:],
                                                    segc[:, k:k + 1], None, ISEQ)
                            nc.tensor.matmul(pagg[:], z_rm[:, ks], ssub[:],
                                             start=(k == 0), stop=(k == K_FIX - 1))

                        agg = sb.tile([128, 128], DT, tag="agg")
                        nc.scalar.copy(agg[:], pagg[:])
                        p1 = pss.tile([128, 128], DT, tag="small")
                        nc.tensor.matmul(p1[:], C[f"Wres1_{l}"][:], agg[:], start=True, stop=True)
                        r1 = sbs.tile([128, 128], DT, tag="r1")
                        nc.vector.scalar_tensor_tensor(r1[:], p1[:], C[f"b_res1_{l}"][:, :1], C["zeros512"][:, :128], ADD, MAX)
                        p2 = pss.tile([128, 128], DT, tag="small")
                        nc.tensor.matmul(p2[:], C[f"Wres2_{l}"][:], r1[:], start=True, stop=True)
                        r2 = sbs.tile([128, 128], DT, tag="r2")
                        nc.vector.scalar_tensor_tensor(r2[:], p2[:], C[f"b_res2_{l}"][:, :1], C["zeros512"][:, :128], ADD, MAX)
                        mnew = sb.tile([128, 128], DT, tag="mnew")
                        nc.vector.tensor_tensor(mnew[:], agg[:], r2[:], op=ADD)
                        nc.vector.tensor_tensor(mnew[:], mnew[:], mt[:, ws], op=ADD)
                        nc.sync.dma_start(dst[:, w * 128:(w + 1) * 128], mnew[:])
                        if l == L - 1:
                            pt = pss.tile([128, 128], DT, tag="small")
                            nc.tensor.transpose(pt[:], mnew[:], C["identity"][:])
                            mrm = sbs.tile([128, 128], DT, tag="mrm")
                            nc.scalar.copy(mrm[:], pt[:])
                            nc.sync.dma_start(msgRM[w * 128:(w + 1) * 128, :], mrm[:])

            # ---------------- phase 2: atom aggregation ----------------
            for w in range([NAW, 0][os.environ.get("SKIP_P2") == "1"]):
                pap = psagg.tile([128, 128], DT, tag="agg")
                srt2 = sbs.tile([128, K_A], mybir.dt.int32, tag="srt")
                nc.sync.dma_start(srt2[:], d_srcrow[:, w * K_A:(w + 1) * K_A])
                tgt2 = sbs.tile([128, K_A], DT, tag="tgt")
                nc.sync.dma_start(tgt2[:], d_tgtrel[:, w * K_A:(w + 1) * K_A])
                for k in range(K_A):
                    gath = sbs.tile([128, 128], DT, tag="gath")
                    nc.gpsimd.indirect_dma_start(
                        out=gath[:], out_offset=None,
                        in_=msgRM[:],
                        in_offset=IndirectOffsetOnAxis(ap=srt2[:, k:k + 1], axis=0))
                    sat = sbs.tile([128, 128], DT, tag="sat")
                    nc.vector.tensor_scalar(sat[:], C["iota_mat"][:], tgt2[:, k:k + 1], None, ISEQ)
                    nc.tensor.matmul(pap[:], gath[:], sat[:],
                                     start=(k == 0), stop=(k == K_A - 1))
                apt = sbs.tile([128, 128], DT, tag="apt")
                nc.scalar.copy(apt[:], pap[:])
                blk = w // (NAW // NCORES)
                col = (w % (NAW // NCORES)) * 128
                nc.sync.dma_start(apart[blk, :, col:col + 128], apt[:])

            if os.environ.get("SKIP_COLL") != "1" and os.environ.get("SKIP_P2") != "1":
                nc.gpsimd.collective_compute(
                    "ReduceScatter", ADD,
                    replica_groups=[list(range(NCORES))],
                    ins=[apart.opt()], outs=[asum.opt()])

            # ---------------- phase 3: output ----------------
            for j in range(ASH // 512):
                cs = slice(j * 512, j * 512 + 512)
                afh = sb.tile([128, 512], DT, tag="mt")
                nc.sync.dma_start(afh[:], d_afT[0:128, cs])
                afl = sbs.tile([FA - 128, 512], DT, tag="afl")
                nc.sync.dma_start(afl[:], d_afT[128:FA, cs])
                ams = sb.tile([128, 512], DT, tag="ret")
                nc.sync.dma_start(ams[:], asum[:, cs])
                po = psb.tile([128, 512], DT, tag="big")
                nc.tensor.matmul(po[:], C["Woa1"][:], afh[:], start=True, stop=False)
                nc.tensor.matmul(po[:], C["Woa2"][:], afl[:], start=False, stop=False)
                nc.tensor.matmul(po[:], C["Wom"][:], ams[:], start=False, stop=True)
                ot = sb.tile([128, 512], DT, tag="s1")
                nc.vector.scalar_tensor_tensor(ot[:], po[:], C["b_o"][:, :1], C["zeros512"][:, :512], ADD, MAX)
                nc.sync.dma_start(d_out[:, cs], ot[:])

    _mark("bass-build")
    nc.compile()
    _mark("nc.compile")

    in_maps = []
    for c in range(NCORES):
        p = per_core[c]
        m = {k: v for k, v in const.items()}
        m.update(ibT=p["ibT"], embiT=p["embiT"], embjT=p["embjT"], dE=p["dE"],
                 envE=p["envE"], tripcat=p["tripcat"], segcolT=p["segcolT"],
                 srcrow=p["srcrow"], tgtrel=p["tgtrel"], afT=p["afT"])
        in_maps.append(m)

    _mark("in-maps")
    res = run_bass_kernel_spmd(nc, in_maps, core_ids=list(range(NCORES)))
    _mark("run")
    global LAST_RESULTS
    LAST_RESULTS = res

    out = np.zeros((N, H), F32)
    for c in range(NCORES):
        lo = c * ASH
        hi = min(N, lo + ASH)
        if hi > lo:
            out[lo:hi] = res.results[c]["outT"][:, :hi - lo].T
    return out



# revision 6
# speedup vs baseline: 3.2514x; 3.2514x over previous
"""DimeNet-style GNN message passing on 8 Trainium2 NeuronCores — v2.

Wall-clock-oriented rewrite of the windowed baseline:
- host does all cheap per-edge GEMMs (BLAS) and vectorized packing
- bulk tensors ship as fp16 (~7MB/core instead of ~30MB/core)
- device program uses For_i hardware loops -> ~600 emitted instructions
  instead of ~16k, collapsing Bass-build + BIR + walrus compile time.

Sharding: edges are packed into 128-edge windows such that each window's
triplet count <= K_FIX*128; windows are dealt to 8 cores.  Gather/scatter
(both keyed by idx_kj) are window-local one-hot matmuls; the only
cross-core communication is one ReduceScatter of [8,H,ASH] atom partials.
"""
import sys
import time as _time
import numpy as np

sys.path.insert(0, "/opt/trn_rl_repo")

H = 128
NR = 16
NS = 6
L = 2
CUTOFF = 8.0
NCORES = 8
TWO_PI = float(2 * np.pi)
F32 = np.float32
F16 = np.float16
LAST_RESULTS = None


def _envelope(x):
    x5 = x ** 5
    return np.where(x < 1.0, 1.0 / x - 28.0 * x5 + 48.0 * x5 * x - 21.0 * x5 * x * x, 0.0)


def _snake(n_items, n_bins):
    pos = np.arange(n_items) % (2 * n_bins)
    return np.where(pos < n_bins, pos, 2 * n_bins - 1 - pos)


def _excl_cumsum(x):
    return np.cumsum(x) - x


def _emulate(per_core, const16, constf, N, EC, NW, K_FIX, TPW, NAW, NA, ASH, K_A):
    """Numpy emulation of the device program (f32; mirrors matmul dataflow)."""
    relu = lambda x: np.maximum(x, 0.0)
    C = {k: v.astype(F32) for k, v in const16.items()}
    C.update(constf)
    TP = NW * TPW
    msgRMs = []
    aparts = []
    for p in per_core:
        rbf0T = p["rbf0T"].astype(F32)
        rbfh = relu(C["Wrbf"].T @ rbf0T + C["b_rbf"])
        ohi = (p["embmeta"][0][None, :] == C["iota_col"]).astype(F32)
        ohj = (p["embmeta"][1][None, :] == C["iota_col"]).astype(F32)
        rbfe = relu(C["tblWi"].T @ ohi + C["tblWj"].T @ ohj
                    + C["Wemb_r"].T @ rbfh + C["b_emb"])
        msg = p["msgT"].astype(F32)
        ang = p["tripmeta"][0]
        seg = p["tripmeta"][1]
        rbf0E = p["rbf0E"].astype(F32)
        for l in range(L):
            kj = relu(C[f"Wkj{l}"].T @ msg + C[f"b_kj{l}"])
            rr = relu(C[f"Wrbf2{l}"].T @ rbfe + C[f"b_rbf2{l}"])
            xkr = kj * rr
            mnew_all = np.zeros_like(msg)
            for w in range(NW):
                es = slice(w * 128, (w + 1) * 128)
                tw = slice(w * TPW, (w + 1) * TPW)
                y = relu(xkr[:, es].T @ C[f"Wdown{l}"] + C[f"bdownr{l}"])
                esub = (seg[tw][None, :] == np.arange(128)[:, None]).astype(F32)
                px = y.T @ esub
                cbf6 = np.cos(np.arange(NS)[:, None] * ang[tw][None, :])
                rtrip = rbf0E[es].T @ esub
                sbf = (C["REP6"].T @ rtrip) * (C["REPC"].T @ cbf6)
                s1 = relu(C[f"Wsbf1{l}"].T @ sbf)
                s2 = relu(C[f"Wsbf2{l}"].T @ s1)
                xs = px * s2
                pagg = np.zeros((128, 128), F32)
                for k in range(K_FIX):
                    ks = slice(k * 128, (k + 1) * 128)
                    zk = relu(xs[:, ks].T @ C[f"Wup{l}"] + C[f"bupr{l}"][:, :128])
                    ssub = (seg[tw][ks][:, None] == np.arange(128)[None, :]).astype(F32)
                    pagg += zk.T @ ssub
                r1 = relu(C[f"Wres1{l}"].T @ pagg + C[f"b_res1{l}"])
                r2 = relu(C[f"Wres2{l}"].T @ r1 + C[f"b_res2{l}"])
                mnew_all[:, es] = pagg + r2 + msg[:, es]
            msg = mnew_all
        msgRMs.append(msg.T.copy())
        apart = np.zeros((NCORES, H, ASH), F32)
        for w in range(NAW):
            gcol = np.zeros((128, 128), F32)
            for k in range(K_A):
                col = w * K_A + k
                gath = msgRMs[-1][p["srcrow"][:, col]]
                sat = (p["tgtrel"][:, col][:, None] == np.arange(128)[None, :]).astype(F32)
                gcol += gath.T @ sat
            blk = w // (NAW // NCORES)
            cc = (w % (NAW // NCORES)) * 128
            apart[blk][:, cc:cc + 128] = gcol
        aparts.append(apart)
    out = np.zeros((N, H), F32)
    for c in range(NCORES):
        asum = np.sum([a[c] for a in aparts], axis=0)
        oc = relu(constf["Wom"].T @ asum + per_core[c]["afWoT"].astype(F32))
        lo = c * ASH
        hi = min(N, lo + ASH)
        out[lo:hi] = oc[:, :hi - lo].T
    return out


def _warm_devices():
    """Touch the axon terminal early: the first device interaction pays the
    whole terminal-attach cost (seconds to minutes under contention), so do
    it in the background while the host packs inputs and compiles."""
    try:
        import jax
        x = jax.device_put(np.zeros((1,), np.float32), jax.devices()[0])
        x.block_until_ready()
    except Exception as e:  # pragma: no cover - warmup is best-effort
        print(f"[kernel] device warmup failed: {e}", file=sys.stderr)


_WARM_THREAD = None


def _start_warm():
    global _WARM_THREAD
    if _WARM_THREAD is None:
        import threading
        _WARM_THREAD = threading.Thread(target=_warm_devices, daemon=True)
        _WARM_THREAD.start()
    return _WARM_THREAD


_start_warm()


def kernel(**inputs):
    _tt = {"t": _time.perf_counter()}

    def _mark(name):
        now = _time.perf_counter()
        print(f"[kernel] {name}: {now - _tt['t']:.2f}s", file=sys.stderr)
        _tt["t"] = now

    _warm_thread = _start_warm()

    import concourse.bass as bass
    import concourse.bacc as bacc
    import concourse.mybir as mybir
    import concourse.tile as tile
    from concourse.bass import IndirectOffsetOnAxis, ds
    from concourse.bass_utils import run_bass_kernel_spmd

    DT = mybir.dt.float32
    DT16 = mybir.dt.float16

    af = np.asarray(inputs["atom_feature"], F32)     # [N,133]
    ef = np.asarray(inputs["edge_feature"], F32)     # [E,14]
    dist = np.asarray(inputs["dist"], F32)           # [E]
    angle = np.asarray(inputs["angle"], F32)         # [T]
    i_idx = np.asarray(inputs["i"]).astype(np.int64)
    j_idx = np.asarray(inputs["j"]).astype(np.int64)
    idx_kj = np.asarray(inputs["idx_kj"]).astype(np.int64)
    ib_eid = np.asarray(inputs["incomebond_edge_ids"]).astype(np.int64)
    ib_atom = np.asarray(inputs["incomebond_index_to_atom"]).astype(np.int64)
    W = {k: np.asarray(v, F32) for k, v in inputs.items()
         if k not in ("atom_feature", "edge_feature", "dist", "angle", "i", "j",
                      "idx_kj", "idx_ji", "incomebond_edge_ids",
                      "incomebond_index_to_atom")}

    N, FA = af.shape
    E = ef.shape[0]
    T = angle.shape[0]

    # ---------------- host per-edge math (BLAS) ----------------
    atom_type = np.argmax(af[:, :100], axis=1)
    d = (dist / CUTOFF).astype(F32)
    env = _envelope(d.astype(np.float64)).astype(F32)
    bf = W["bessel_freq"]                            # [16] = pi*(1..16)
    rbf0 = env[:, None] * np.sin(bf[None, :] * d[:, None])        # [E,16]
    afW = af @ W["W_i1_w"][:FA]
    efW = ef @ W["W_i1_w"][FA:]
    msg0 = np.maximum(afW[j_idx] + efW + W["W_i1_b"], 0.0)        # [E,H]
    tblWi = np.zeros((128, H), F32)
    tblWj = np.zeros((128, H), F32)
    tblWi[:100] = W["emb_table"] @ W["lin_emb_w"][:H]
    tblWj[:100] = W["emb_table"] @ W["lin_emb_w"][H:2 * H]
    afWo = af @ W["W_o_w"][:FA] + W["W_o_b"]                      # [N,H]
    type_i = atom_type[i_idx].astype(F32)
    type_j = atom_type[j_idx].astype(F32)

    # ---------------- edge -> window packing ----------------
    deg = np.bincount(idx_kj, minlength=E)
    order = np.argsort(-deg, kind="stable")
    NW_TOT = -(-(-(-E // 128)) // 32) * 32
    while T / NW_TOT > 490.0:
        NW_TOT += 32
    while True:
        w_of = _snake(E, NW_TOT)                     # window of rank k
        wload = np.bincount(w_of, weights=deg[order].astype(np.float64),
                            minlength=NW_TOT).astype(np.int64)
        K_FIX = max(1, -(-int(wload.max()) // 128))
        cnt_w = np.bincount(w_of, minlength=NW_TOT)
        if K_FIX <= 4 and cnt_w.max() <= 128:
            break
        NW_TOT += 32                                 # repack smaller windows
    grouped = order[np.argsort(w_of, kind="stable")]
    cum_w = np.concatenate([[0], np.cumsum(cnt_w)])
    TPW = 128 * K_FIX
    NW = NW_TOT // NCORES
    EC = NW * 128
    NSC = EC // 512
    TP = NW * TPW

    worder = np.argsort(-wload, kind="stable")
    core_snake = _snake(NW_TOT, NCORES)

    t_order = np.argsort(idx_kj, kind="stable")
    t_sorted_edge = idx_kj[t_order]
    seg_starts = np.searchsorted(t_sorted_edge, np.arange(E))

    # global slot axis over all cores: core c owns slots [c*EC, (c+1)*EC)
    wlist_all = np.concatenate([worder[core_snake == c] for c in range(NCORES)])
    starts_g = cum_w[wlist_all]
    lens_g = cnt_w[wlist_all]
    Lg = int(lens_g.sum())
    assert Lg == E
    within_g = np.arange(Lg) - np.repeat(_excl_cumsum(lens_g), lens_g)
    src_g = np.repeat(starts_g, lens_g) + within_g
    slots_g = np.repeat(np.arange(NCORES * NW) * 128, lens_g) + within_g
    edge_ids = np.full(NCORES * EC, -1, np.int64)
    edge_ids[slots_g] = grouped[src_g]
    real = edge_ids >= 0
    re = edge_ids[real]
    owner = np.empty(E, np.int32)
    localrow = np.empty(E, np.int32)
    sl_real = np.nonzero(real)[0]
    owner[re] = (sl_real // EC).astype(np.int32)
    localrow[re] = (sl_real % EC).astype(np.int32)

    msgT_g = np.zeros((H, NCORES * EC), F16)
    msgT_g[:, real] = msg0[re].T
    rbf0T_g = np.zeros((NR, NCORES * EC), F16)
    rbf016 = rbf0.astype(F16)
    rbf0T_g[:, real] = rbf016[re].T
    rbf0E_g = np.zeros((NCORES * EC, NR), F16)
    rbf0E_g[real] = rbf016[re]
    embmeta_g = np.zeros((2, NCORES * EC), F32)
    embmeta_g[0, real] = type_i[re]
    embmeta_g[1, real] = type_j[re]

    # triplet slots (global): window-local cumsum of per-slot triplet counts
    ndeg = np.where(real, deg[np.maximum(edge_ids, 0)], 0)
    c2 = np.cumsum(ndeg)
    wsc = np.concatenate([[0], c2[127::128][:-1]])   # excl cumsum at window starts
    win_of_slot = np.arange(NCORES * EC) // 128
    start_of_slot = win_of_slot * TPW + (c2 - ndeg - wsc[win_of_slot])
    sel = ndeg > 0
    lens2 = ndeg[sel]
    Tc = int(lens2.sum())
    within2 = np.arange(Tc) - np.repeat(_excl_cumsum(lens2), lens2)
    src_rank = np.repeat(seg_starts[edge_ids[sel]], lens2) + within2
    t_ids = t_order[src_rank]
    dest = np.repeat(start_of_slot[sel], lens2) + within2
    ang_g = np.zeros(NCORES * TP, F32)
    ang_g[dest] = angle[t_ids]
    seg_g = np.full(NCORES * TP, -1.0, F32)
    seg_g[dest] = np.repeat(np.arange(NCORES * EC)[sel] % 128, lens2).astype(F32)

    per_core = []
    for c in range(NCORES):
        es = slice(c * EC, (c + 1) * EC)
        ts = slice(c * TP, (c + 1) * TP)
        seg_c = seg_g[ts]
        per_core.append(dict(
            msgT=msgT_g[:, es], rbf0T=rbf0T_g[:, es], rbf0E=rbf0E_g[es],
            embmeta=embmeta_g[:, es],
            tripmeta=np.ascontiguousarray(
                np.stack([ang_g[ts], seg_c])),
            segcolT=np.ascontiguousarray(seg_c.reshape(NW * K_FIX, 128).T)))

    # ---------------- income bonds (needs owner/localrow complete) ----------
    NAW = -(-(-(-N // 128)) // 32) * 32
    NA = NAW * 128
    ASH = NA // NCORES
    NAB = NAW // NCORES                              # atom windows per block
    bond_owner = owner[ib_eid]
    aw_all = ib_atom // 128
    bucket = bond_owner.astype(np.int64) * NAW + aw_all
    cnts = np.bincount(bucket, minlength=NCORES * NAW)
    K_A = max(1, -(-int(cnts.max()) // 128))
    BPW = 128 * K_A
    o2 = np.argsort(bucket, kind="stable")
    within = np.arange(E) - np.repeat(_excl_cumsum(cnts), cnts)
    destb = bucket[o2] * BPW + within
    srcflat = np.zeros(NCORES * NAW * BPW, np.int32)
    srcflat[destb] = localrow[ib_eid[o2]]
    tgtflat = np.full(NCORES * NAW * BPW, -1.0, F32)
    tgtflat[destb] = (ib_atom[o2] - aw_all[o2] * 128).astype(F32)
    for c in range(NCORES):
        bs = slice(c * NAW * BPW, (c + 1) * NAW * BPW)
        per_core[c]["srcrow"] = np.ascontiguousarray(
            srcflat[bs].reshape(NAW * K_A, 128).T)
        per_core[c]["tgtrel"] = np.ascontiguousarray(
            tgtflat[bs].reshape(NAW * K_A, 128).T)
        afWoT = np.zeros((H, ASH), F16)
        lo = c * ASH
        hi = min(N, lo + ASH)
        afWoT[:, :hi - lo] = afWo[lo:hi].T
        per_core[c]["afWoT"] = afWoT

    # ---------------- replicated constants ----------------
    const16 = dict(
        tblWi=tblWi, tblWj=tblWj,
        Wrbf=W["lin_rbf_w"], Wemb_r=W["lin_emb_w"][2 * H:],
        REP6=np.tile(np.eye(NR, dtype=F32), (1, NS)),
        REPC=np.repeat(np.eye(NS, dtype=F32), NR, axis=1),
        ident=np.eye(128, dtype=F32),
        ones16=np.ones((1, 512), F32),
    )
    constf = dict(
        onesf=np.ones((1, 512), F32),
        q025=np.full((1, NS), 0.25, F32),
        svecn=(np.arange(NS, dtype=F32) / TWO_PI).reshape(1, NS),
        iota_col=np.arange(128, dtype=F32).reshape(128, 1),
        iota_mat=np.tile(np.arange(128, dtype=F32), (128, 1)),
        b_emb=W["lin_emb_b"].reshape(H, 1),
        b_rbf=W["lin_rbf_b"].reshape(H, 1),
        Wom=W["W_o_w"][FA:],
    )
    for l in range(L):
        const16[f"Wkj{l}"] = W["L_kj_w"][l]
        const16[f"Wrbf2{l}"] = W["L_rbf2_w"][l]
        const16[f"Wsbf1{l}"] = W["L_sbf1_w"][l]
        const16[f"Wsbf2{l}"] = W["L_sbf2_w"][l]
        const16[f"Wdown{l}"] = W["L_down_w"][l]
        const16[f"Wup{l}"] = W["L_up_w"][l]
        const16[f"Wres1{l}"] = W["L_res1_w"][l]
        const16[f"Wres2{l}"] = W["L_res2_w"][l]
        const16[f"bdownr{l}"] = W["L_down_b"][l].reshape(1, H)
        const16[f"bupr{l}"] = np.tile(W["L_up_b"][l].reshape(1, H), (1, K_FIX))
        constf[f"b_kj{l}"] = W["L_kj_b"][l].reshape(H, 1)
        constf[f"b_rbf2{l}"] = W["L_rbf2_b"][l].reshape(H, 1)
        constf[f"b_res1{l}"] = W["L_res1_b"][l].reshape(H, 1)
        constf[f"b_res2{l}"] = W["L_res2_b"][l].reshape(H, 1)
    const16 = {k: v.astype(F16) for k, v in const16.items()}

    _mark("host-prep")

    import os
    if os.environ.get("EMU") == "1":
        return _emulate(per_core, const16, constf, N, EC, NW, K_FIX, TPW,
                        NAW, NA, ASH, K_A)

    # ------------------------------------------------------------------
    # Bass program (identical on all cores)
    # ------------------------------------------------------------------
    nc = bacc.Bacc("TRN2", target_bir_lowering=False, debug=False,
                   num_devices=NCORES)

    def din(name, arr, dt):
        return nc.dram_tensor(name, list(arr.shape), dt, kind="ExternalInput")

    d_c16 = {k: din(k, v, DT16) for k, v in const16.items()}
    d_cf = {k: din(k, v, DT) for k, v in constf.items()}
    p0 = per_core[0]
    d_msgT = din("msgT", p0["msgT"], DT16)
    d_rbf0T = din("rbf0T", p0["rbf0T"], DT16)
    d_rbf0E = din("rbf0E", p0["rbf0E"], DT16)
    d_embmeta = din("embmeta", p0["embmeta"], DT)
    d_tripmeta = din("tripmeta", p0["tripmeta"], DT)
    d_segcolT = din("segcolT", p0["segcolT"], DT)
    d_srcrow = nc.dram_tensor("srcrow", list(p0["srcrow"].shape),
                              mybir.dt.int32, kind="ExternalInput")
    d_tgtrel = din("tgtrel", p0["tgtrel"], DT)
    d_afWoT = din("afWoT", p0["afWoT"], DT16)
    d_out = nc.dram_tensor("outT", [H, ASH], DT16, kind="ExternalOutput")

    RELU = mybir.ActivationFunctionType.Relu
    SIN = mybir.ActivationFunctionType.Sin
    ADD = mybir.AluOpType.add
    MULT = mybir.AluOpType.mult
    ISEQ = mybir.AluOpType.is_equal
    MAX = mybir.AluOpType.max

    with tile.TileContext(nc) as tc:
        with (
            tc.tile_pool(name="const", bufs=1) as cpool,
            tc.tile_pool(name="sb", bufs=3) as sb,
            tc.tile_pool(name="sbs", bufs=3) as sbs,
            tc.tile_pool(name="psb", bufs=3, space="PSUM") as psb,
            tc.tile_pool(name="pss", bufs=3, space="PSUM") as pss,
            tc.tile_pool(name="psagg", bufs=2, space="PSUM") as psagg,
            tc.tile_pool(name="dram", bufs=1, space="DRAM") as dram,
        ):
            C = {}
            for k, v in const16.items():
                t = cpool.tile(list(v.shape), DT16, tag=k)
                nc.sync.dma_start(t[:], d_c16[k][:])
                C[k] = t
            for k, v in constf.items():
                t = cpool.tile(list(v.shape), DT, tag=k)
                nc.sync.dma_start(t[:], d_cf[k][:])
                C[k] = t

            msgA = dram.tile([H, EC], DT16, tag="msgA")
            msgB = dram.tile([H, EC], DT16, tag="msgB")
            rbfeT = dram.tile([H, EC], DT16, tag="rbfeT")
            msgRM = dram.tile([EC, H], DT16, tag="msgRM")
            apart = dram.tile([NCORES, H, ASH], DT, tag="apart")
            asum = dram.tile([H, ASH], DT, tag="asum")

            def sin_eval(p_arg, parts, width):
                """p_arg PSUM holds arg/(2pi) >= 0; returns fp16 sin(arg)."""
                qi = sbs.tile([parts, width], mybir.dt.int32, tag="sinqi")
                nc.vector.tensor_copy(qi[:], p_arg[:])
                qf = sbs.tile([parts, width], DT, tag="sinqf")
                nc.vector.tensor_copy(qf[:], qi[:])
                y = sbs.tile([parts, width], DT, tag="siny")
                nc.vector.scalar_tensor_tensor(y[:], qf[:], -1.0, p_arg[:], MULT, ADD)
                s = sbs.tile([parts, width], DT16, tag="sins")
                nc.scalar.activation(s[:], y[:], SIN, scale=TWO_PI)
                return s

            # ---------------- phase 0: rbf_e ----------------
            with tc.For_i(0, NSC) as it:
                cs = ds(it * 512, 512)
                meta_i = sbs.tile([1, 512], DT, tag="meta_i")
                nc.sync.dma_start(meta_i[:], d_embmeta[0:1, cs])
                meta_j = sbs.tile([1, 512], DT, tag="meta_j")
                nc.sync.dma_start(meta_j[:], d_embmeta[1:2, cs])
                r0 = sbs.tile([NR, 512], DT16, tag="r0")
                nc.sync.dma_start(r0[:], d_rbf0T[:, cs])
                prh = pss.tile([128, 512], DT, tag="small")
                nc.tensor.matmul(prh[:], C["Wrbf"][:], r0[:], start=True, stop=True)
                rbfh = sb.tile([128, 512], DT16, tag="rbfh")
                nc.scalar.activation(rbfh[:], prh[:], RELU, bias=C["b_rbf"][:, :1])
                bi = sb.tile([128, 512], DT, tag="bi")
                nc.gpsimd.partition_broadcast(bi[:], meta_i[:])
                ohi = sb.tile([128, 512], DT16, tag="ohi")
                nc.vector.tensor_scalar(ohi[:], bi[:], C["iota_col"][:, :1], None, ISEQ)
                bj = sb.tile([128, 512], DT, tag="bj")
                nc.gpsimd.partition_broadcast(bj[:], meta_j[:])
                ohj = sb.tile([128, 512], DT16, tag="ohj")
                nc.vector.tensor_scalar(ohj[:], bj[:], C["iota_col"][:, :1], None, ISEQ)
                pre = psb.tile([128, 512], DT, tag="big")
                nc.tensor.matmul(pre[:], C["tblWi"][:], ohi[:], start=True, stop=False)
                nc.tensor.matmul(pre[:], C["tblWj"][:], ohj[:], start=False, stop=False)
                nc.tensor.matmul(pre[:], C["Wemb_r"][:], rbfh[:], start=False, stop=True)
                rbe = sb.tile([128, 512], DT16, tag="rbe")
                nc.vector.tensor_scalar(rbe[:], pre[:], C["b_emb"][:, :1], 0.0, ADD, MAX)
                nc.sync.dma_start(rbfeT[:, cs], rbe[:])

            # ---------------- phase 1: interaction layers ----------------
            for l in range(L):
                src = d_msgT if l == 0 else (msgB if l == 1 else msgA)
                dst = msgB if l == 0 else msgA
                with tc.For_i(0, NSC) as it:
                    cs = ds(it * 512, 512)
                    mt = sb.tile([128, 512], DT16, tag="mt")
                    nc.sync.dma_start(mt[:], src[:, cs])
                    ret = sb.tile([128, 512], DT16, tag="ret")
                    nc.sync.dma_start(ret[:], rbfeT[:, cs])
                    pkj = psb.tile([128, 512], DT, tag="big")
                    nc.tensor.matmul(pkj[:], C[f"Wkj{l}"][:], mt[:], start=True, stop=True)
                    kj = sb.tile([128, 512], DT16, tag="kj")
                    nc.vector.tensor_scalar(kj[:], pkj[:], C[f"b_kj{l}"][:, :1], 0.0, ADD, MAX)
                    pr = psb.tile([128, 512], DT, tag="big")
                    nc.tensor.matmul(pr[:], C[f"Wrbf2{l}"][:], ret[:], start=True, stop=True)
                    rr = sb.tile([128, 512], DT16, tag="rr")
                    nc.scalar.activation(rr[:], pr[:], RELU, bias=C[f"b_rbf2{l}"][:, :1])
                    xkr = sb.tile([128, 512], DT16, tag="xkr")
                    nc.vector.tensor_tensor(xkr[:], kj[:], rr[:], op=MULT)

                    tma = sbs.tile([1, 4 * TPW], DT, tag="tma")
                    nc.sync.dma_start(tma[:], d_tripmeta[0:1, ds(it * (4 * TPW), 4 * TPW)])
                    tms = sbs.tile([1, 4 * TPW], DT, tag="tms")
                    nc.sync.dma_start(tms[:], d_tripmeta[1:2, ds(it * (4 * TPW), 4 * TPW)])
                    sc4 = sbs.tile([128, 4 * K_FIX], DT, tag="sc4")
                    nc.sync.dma_start(sc4[:], d_segcolT[:, ds(it * (4 * K_FIX), 4 * K_FIX)])

                    for wi in range(4):
                        tw = slice(wi * TPW, (wi + 1) * TPW)
                        es = slice(wi * 128, (wi + 1) * 128)
                        rE = sbs.tile([128, NR], DT16, tag="rE", bufs=2)
                        nc.sync.dma_start(rE[:], d_rbf0E[ds((it * 4 + wi) * 128, 128), :])
                        segb = sb.tile([128, TPW], DT, tag="segb", bufs=2)
                        nc.gpsimd.partition_broadcast(segb[:], tms[:, tw])
                        esub = sb.tile([128, TPW], DT16, tag="esub", bufs=2)
                        nc.vector.tensor_scalar(esub[:], segb[:], C["iota_col"][:, :1], None, ISEQ)

                        # sbf = (REPC@cbf) * (REP6@(rbf0E expanded))
                        pa = pss.tile([NS, TPW], DT, tag="small")
                        nc.tensor.matmul(pa[:], C["q025"][:], C["onesf"][:, :TPW],
                                         start=True, stop=False)
                        nc.tensor.matmul(pa[:], C["svecn"][:], tma[:, tw],
                                         start=False, stop=True)
                        cbf6 = sin_eval(pa, NS, TPW)
                        p16 = pss.tile([NR, TPW], DT, tag="small")
                        nc.tensor.matmul(p16[:], rE[:], esub[:], start=True, stop=True)
                        c16 = sbs.tile([NR, TPW], DT16, tag="c16")
                        nc.scalar.copy(c16[:], p16[:])
                        p96r = psb.tile([NS * NR, TPW], DT, tag="big")
                        nc.tensor.matmul(p96r[:], C["REP6"][:], c16[:], start=True, stop=True)
                        p96c = psb.tile([NS * NR, TPW], DT, tag="big")
                        nc.tensor.matmul(p96c[:], C["REPC"][:], cbf6[:], start=True, stop=True)
                        c96 = sb.tile([NS * NR, TPW], DT16, tag="c96")
                        nc.scalar.copy(c96[:], p96c[:])
                        sbf = sb.tile([NS * NR, TPW], DT16, tag="sbf")
                        nc.vector.tensor_tensor(sbf[:], p96r[:], c96[:], op=MULT)
                        ps1 = psb.tile([128, TPW], DT, tag="big")
                        nc.tensor.matmul(ps1[:], C[f"Wsbf1{l}"][:], sbf[:], start=True, stop=True)
                        s1 = sb.tile([128, TPW], DT16, tag="s1")
                        nc.scalar.activation(s1[:], ps1[:], RELU)
                        ps2 = psb.tile([128, TPW], DT, tag="big")
                        nc.tensor.matmul(ps2[:], C[f"Wsbf2{l}"][:], s1[:], start=True, stop=True)
                        s2 = sb.tile([128, TPW], DT16, tag="s2")
                        nc.scalar.activation(s2[:], ps2[:], RELU)

                        # down-projection (edge-major), expand, multiply, up
                        py = pss.tile([128, 128], DT, tag="small")
                        nc.tensor.matmul(py[:], C["ones16"][:, :128], C[f"bdownr{l}"][:],
                                         start=True, stop=False)
                        nc.tensor.matmul(py[:], xkr[:, es], C[f"Wdown{l}"][:],
                                         start=False, stop=True)
                        y = sb.tile([128, 128], DT16, tag="y")
                        nc.scalar.activation(y[:], py[:], RELU)
                        px = psb.tile([128, TPW], DT, tag="big")
                        nc.tensor.matmul(px[:], y[:], esub[:], start=True, stop=True)
                        xs = sb.tile([128, TPW], DT16, tag="xs")
                        nc.vector.tensor_tensor(xs[:], px[:], s2[:], op=MULT)
                        pz = psb.tile([128, TPW], DT, tag="big")
                        nc.tensor.matmul(pz[:], C["ones16"][:, :128], C[f"bupr{l}"][:],
                                         start=True, stop=False)
                        for k in range(K_FIX):
                            ks = slice(k * 128, (k + 1) * 128)
                            nc.tensor.matmul(pz[:, ks], xs[:, ks], C[f"Wup{l}"][:],
                                             start=False, stop=(k == K_FIX - 1))
                        z = sb.tile([128, TPW], DT16, tag="z")
                        nc.vector.tensor_scalar(z[:], pz[:], 0.0, None, MAX)

                        pagg = psagg.tile([128, 128], DT, tag="agg")
                        for k in range(K_FIX):
                            ks = slice(k * 128, (k + 1) * 128)
                            ssub = sbs.tile([128, 128], DT16, tag="ssub")
                            nc.vector.tensor_scalar(ssub[:], C["iota_mat"][:],
                                                    sc4[:, wi * K_FIX + k:wi * K_FIX + k + 1],
                                                    None, ISEQ)
                            nc.tensor.matmul(pagg[:], z[:, ks], ssub[:],
                                             start=(k == 0), stop=(k == K_FIX - 1))
                        agg = sb.tile([128, 128], DT16, tag="agg")
                        nc.scalar.copy(agg[:], pagg[:])
                        p1 = pss.tile([128, 128], DT, tag="small")
                        nc.tensor.matmul(p1[:], C[f"Wres1{l}"][:], agg[:], start=True, stop=True)
                        r1 = sbs.tile([128, 128], DT16, tag="r1")
                        nc.vector.tensor_scalar(r1[:], p1[:], C[f"b_res1{l}"][:, :1], 0.0, ADD, MAX)
                        p2 = pss.tile([128, 128], DT, tag="small")
                        nc.tensor.matmul(p2[:], C[f"Wres2{l}"][:], r1[:], start=True, stop=True)
                        r2 = sbs.tile([128, 128], DT16, tag="r2")
                        nc.scalar.activation(r2[:], p2[:], RELU, bias=C[f"b_res2{l}"][:, :1])
                        mnew = sb.tile([128, 128], DT16, tag="mnew")
                        nc.vector.tensor_tensor(mnew[:], agg[:], r2[:], op=ADD)
                        nc.vector.tensor_tensor(mnew[:], mnew[:], mt[:, es], op=ADD)
                        if l < L - 1:
                            nc.sync.dma_start(dst[:, ds((it * 4 + wi) * 128, 128)], mnew[:])
                        if l == L - 1:
                            pt = pss.tile([128, 128], DT16, tag="small")
                            nc.tensor.transpose(pt[:], mnew[:], C["ident"][:])
                            mrm = sbs.tile([128, 128], DT16, tag="mrm")
                            nc.scalar.copy(mrm[:], pt[:])
                            nc.sync.dma_start(msgRM[ds((it * 4 + wi) * 128, 128), :], mrm[:])

            # ---------------- phase 2: atom aggregation ----------------
            for blk in range(NCORES):
                with tc.For_i(0, NAB) as wt:
                    wk = (blk * NAB) * K_A + wt * K_A
                    srt = sbs.tile([128, K_A], mybir.dt.int32, tag="srt")
                    nc.sync.dma_start(srt[:], d_srcrow[:, ds(wk, K_A)])
                    tgt = sbs.tile([128, K_A], DT, tag="tgt")
                    nc.sync.dma_start(tgt[:], d_tgtrel[:, ds(wk, K_A)])
                    pap = psagg.tile([128, 128], DT, tag="agg")
                    for k in range(K_A):
                        gath = sbs.tile([128, 128], DT16, tag="gath")
                        nc.gpsimd.indirect_dma_start(
                            out=gath[:], out_offset=None,
                            in_=msgRM[:],
                            in_offset=IndirectOffsetOnAxis(ap=srt[:, k:k + 1], axis=0))
                        sat = sbs.tile([128, 128], DT16, tag="sat")
                        nc.vector.tensor_scalar(sat[:], C["iota_mat"][:],
                                                tgt[:, k:k + 1], None, ISEQ)
                        nc.tensor.matmul(pap[:], gath[:], sat[:],
                                         start=(k == 0), stop=(k == K_A - 1))
                    apt = sbs.tile([128, 128], DT, tag="apt")
                    nc.scalar.copy(apt[:], pap[:])
                    nc.sync.dma_start(apart[blk, :, ds(wt * 128, 128)], apt[:])

            nc.gpsimd.collective_compute(
                "ReduceScatter", ADD,
                replica_groups=[list(range(NCORES))],
                ins=[apart.opt()], outs=[asum.opt()])

            # ---------------- phase 3: output ----------------
            for jc in range(ASH // 512):
                cs = slice(jc * 512, (jc + 1) * 512)
                ams = sb.tile([128, 512], DT, tag="ams")
                nc.sync.dma_start(ams[:], asum[:, cs])
                ao = sb.tile([128, 512], DT16, tag="ao")
                nc.sync.dma_start(ao[:], d_afWoT[:, cs])
                po = psb.tile([128, 512], DT, tag="big")
                nc.tensor.matmul(po[:], C["Wom"][:], ams[:], start=True, stop=True)
                t1 = sb.tile([128, 512], DT, tag="t1")
                nc.vector.tensor_tensor(t1[:], po[:], ao[:], op=ADD)
                ot = sb.tile([128, 512], DT16, tag="ot")
                nc.vector.tensor_scalar(ot[:], t1[:], 0.0, None, MAX)
                nc.sync.dma_start(d_out[:, cs], ot[:])

    _mark("bass-build")
    nc.compile()
    _mark("nc.compile")

    in_maps = []
    for c in range(NCORES):
        p = per_core[c]
        m = {}
        m.update(const16)
        m.update(constf)
        m.update(msgT=p["msgT"], rbf0T=p["rbf0T"], rbf0E=p["rbf0E"],
                 embmeta=p["embmeta"], tripmeta=p["tripmeta"],
                 segcolT=p["segcolT"], srcrow=p["srcrow"], tgtrel=p["tgtrel"],
                 afWoT=p["afWoT"])
        in_maps.append(m)

    _mark("in-maps")
    _warm_thread.join()
    _mark("warm-join")
    res = run_bass_kernel_spmd(nc, in_maps, core_ids=list(range(NCORES)))
    _mark("run")
    global LAST_RESULTS
    LAST_RESULTS = res

    out = np.zeros((N, H), F32)
    for c in range(NCORES):
        lo = c * ASH
        hi = min(N, lo + ASH)
        if hi > lo:
            out[lo:hi] = res.results[c]["outT"][:, :hi - lo].T.astype(F32)
    return out
